# revision 40
# baseline (speedup 1.0000x reference)
"""Trainium2 Bass kernel for a single causal attention head.

  x:  [32, 1024, 768] f32, Wq/Wk/Wv: [64, 768] f32
  out[b,q,:] = softmax_k(causal(Q K^T / 8)) @ V,  Q = x Wq^T etc.

Sharding: data-parallel over batch — 4 batches per core on 8 cores,
weights replicated.

The whole kernel runs in bf16 (measured end-to-end rel err ~6e-3 vs the
fp32 reference, threshold 2e-2). The host casts x to bf16 and pre-packs
the weight stationaries, which buys:

  1. Half the HBM traffic for x, and the transposition of x (contraction
     dim c must sit on partitions for the projection matmuls) rides the
     DMA xbar transpose engine (16-bit only) instead of costing ~190
     LDWEIGHTS+MATMUL pairs on the PE like the f32r version did. The
     xbar's per-queue issue rate (~1.8us per [1024,128] chunk) is the
     scarce resource, so chunks are split across both HWDGE queues
     (sync/scalar) with the scalar queue's share front-loaded before its
     exp work begins.
  2. bf16 matmul moving operands stream at 1 cycle/row at ANY width
     (f32r needs >=256 cols), so the short causal attention segments
     stop paying a 4x penalty, and LDWEIGHTS gets the fast-weight-load
     path.

per batch b (all on one NeuronCore):
  - xT [c=128, chunk j, t=1024] via 6 DMA xbar transposes.
  - two packed projection passes per 512-col half:
    [Wq^T|Wk^T] -> [Q^T; K^T] and [Wv^T|Wq^T] -> [V^T; Q^T] (the Q^T
    copy lands on partitions 64:128 where the S^T matmul needs its
    moving operand).
  - S^T = K^T.T Q^T per 128-row k-block (causal blocks only), additive
    -1e9 mask on the diagonal block (DVE), exp on the scalar engine.
  - AV in q-major orientation: stationary = E k-block chunk, moving =
    [V | ones] tile, accumulating out[q-tile, 65] directly in PSUM. The
    ones column makes col 64 the softmax denominator, and q-major means
    no output transposes or PSUM->SBUF copy chain — DVE normalizes
    straight out of PSUM.

Batches are software-pipelined: attention of batch b-1 interleaves with
the projection chunks of batch b so the PE never idles long enough for
the HAM clock gate to re-throttle; junk matmuls warm the clock during
the initial DMA fill.
"""

import os
import sys
import numpy as np

B_FULL = 32
N_CORES = 8
B_CORE = B_FULL // N_CORES
T, C, D = 1024, 768, 64
TT = T // 128  # 8
CC = C // 128  # 6
SCALE = 1.0 / np.sqrt(D)

_cache = {}


def _seg512(q0, q1):
    """Split [q0, q1) at 512 boundaries (PSUM bank limit)."""
    segs = []
    while q0 < q1:
        q_end = min(q1, (q0 // 512 + 1) * 512)
        segs.append((q0, q_end))
        q0 = q_end
    return segs


def _build():
    from contextlib import ExitStack

    import concourse.bass as bass
    import concourse.tile as tile
    from concourse import bacc, mybir
    from concourse.bass import ts
    from concourse.masks import make_identity

    f32 = mybir.dt.float32
    bf = mybir.dt.bfloat16
    nc = bacc.Bacc("TRN2", target_bir_lowering=False, debug=False)
    x = nc.dram_tensor("x", [B_CORE, T, C], bf, kind="ExternalInput").ap()
    wqk = nc.dram_tensor("wqk", [128, CC, 128], bf, kind="ExternalInput").ap()
    wvq = nc.dram_tensor("wvq", [128, CC, 128], bf, kind="ExternalInput").ap()
    y = nc.dram_tensor("y", [B_CORE, T, D], f32, kind="ExternalOutput").ap()

    with tile.TileContext(nc) as tc, ExitStack() as ctx:
        const = ctx.enter_context(tc.tile_pool(name="const", bufs=1))
        xtp = ctx.enter_context(tc.tile_pool(name="xt", bufs=4))
        sb = ctx.enter_context(tc.tile_pool(name="sb", bufs=2))
        epool = ctx.enter_context(tc.tile_pool(name="e", bufs=4))
        ypool = ctx.enter_context(tc.tile_pool(name="yout", bufs=4))
        # PSUM: 8 banks of [128 x 2KB]:
        #   ps_proj: 2 x [128,512] f32 (qk / vq half accums)     = 2 banks
        #   ps_st:   2 x [128,512] f32 (S^T chunks)              = 2 banks
        #   ps_vp:   2 x [128,4,65] bf16 ([V|1] tile transposes) = 2 banks
        #   ps_out:  2 x [128,4,65] f32 (q-major AV accum)       = 2 banks
        ps_proj = ctx.enter_context(tc.tile_pool(name="ps_proj", bufs=2, space="PSUM"))
        ps_st = ctx.enter_context(tc.tile_pool(name="ps_st", bufs=2, space="PSUM"))
        ps_vp = ctx.enter_context(tc.tile_pool(name="ps_vp", bufs=2, space="PSUM"))
        ps_out = ctx.enter_context(tc.tile_pool(name="ps_out", bufs=2, space="PSUM"))

        # ---- constants ----
        WQK = const.tile([128, CC, 128], bf, tag="wqk")
        WVQ = const.tile([128, CC, 128], bf, tag="wvq")
        nc.sync.dma_start(WQK, wqk)
        nc.sync.dma_start(WVQ, wvq)
        ident = const.tile([128, 128], f32, tag="ident")
        make_identity(nc, ident)
        ident_b = const.tile([128, 128], bf, tag="ident_b")
        nc.gpsimd.tensor_copy(ident_b, ident)
        # additive causal mask for the S^T diagonal block: 0 where k<=q
        # (p<=f), -1e9 where k>q
        dmask = const.tile([128, 128], f32, tag="dmask")
        nc.gpsimd.memset(dmask, 0.0)
        nc.gpsimd.affine_select(
            out=dmask,
            in_=dmask,
            compare_op=mybir.AluOpType.is_ge,
            fill=-1e9,
            base=0,
            pattern=[[1, 128]],
            channel_multiplier=-1,
        )

        states = {b: {"b": b, "E": {}} for b in range(B_CORE)}

        def op_xt(b, j, eng):
            """DMA xbar transpose of one [1024,128] c-chunk of x[b]."""
            st8 = states[b]
            if "xT" not in st8:
                st8["xT"] = xtp.tile([128, CC, T], bf, tag="xT", name=f"xT{b}")
            eng.dma_start(st8["xT"][:, j, :], x[b][:, ts(j, 128)], transpose=True)

        def op_proj_h0_j(b, j):
            st8 = states[b]
            if j == 0:
                st8["QK_sb"] = sb.tile([128, T], bf, tag="qksb", name="QK_sb")
                st8["Qd_sb"] = sb.tile([128, T], bf, tag="qd", name="Qd_sb")
                st8["VT_sb"] = sb.tile([128, T], bf, tag="vtsb", name="VT_sb")
                st8["qk_h0"] = ps_proj.tile([128, 512], f32, tag="ps_proj", name="qk_h0")
                st8["vq_h0"] = ps_proj.tile([128, 512], f32, tag="ps_proj", name="vq_h0")
            for W, ps in ((WQK, st8["qk_h0"]), (WVQ, st8["vq_h0"])):
                nc.tensor.matmul(
                    ps,
                    W[:, j, :],
                    st8["xT"][:, j, 0:512],
                    start=(j == 0),
                    stop=(j == CC - 1),
                )

        def _stage_half(st8, h, qk_ps, vq_ps):
            hs = ts(h, 512)
            nc.vector.tensor_copy(st8["QK_sb"][:, hs], qk_ps)
            # Q^T dup: psum rows 64:128 -> SBUF rows 64:128 (same partitions)
            nc.vector.tensor_copy(st8["Qd_sb"][64:128, hs], vq_ps[64:128, :])
            nc.vector.tensor_copy(st8["VT_sb"][0:64, hs], vq_ps[0:64, :])

        def op_proj_h0_stage(b):
            st8 = states[b]
            _stage_half(st8, 0, st8.pop("qk_h0"), st8.pop("vq_h0"))

        def op_proj_h1(b):
            st8 = states[b]
            qk_ps = ps_proj.tile([128, 512], f32, tag="ps_proj", name="qk_ps")
            vq_ps = ps_proj.tile([128, 512], f32, tag="ps_proj", name="vq_ps")
            for W, ps in ((WQK, qk_ps), (WVQ, vq_ps)):
                for j in range(CC):
                    nc.tensor.matmul(
                        ps,
                        W[:, j, :],
                        st8["xT"][:, j, 512:1024],
                        start=(j == 0),
                        stop=(j == CC - 1),
                    )
            st8.pop("xT")
            _stage_half(st8, 1, qk_ps, vq_ps)
            # ones row for the softmax denominator column
            nc.gpsimd.memset(st8["VT_sb"][64:65, :], 1.0)

        def op_proj0_j(j):
            """Batch-0-only: both 512-col halves per chunk. The h1
            accumulators borrow the ps_st banks (no S^T work exists yet),
            doubling step-0 PE duty and killing the dense h1 tail that
            otherwise runs cold right before the first attention phase."""
            st8 = states[0]
            if j == 0:
                st8["QK_sb"] = sb.tile([128, T], bf, tag="qksb", name="QK_sb")
                st8["Qd_sb"] = sb.tile([128, T], bf, tag="qd", name="Qd_sb")
                st8["VT_sb"] = sb.tile([128, T], bf, tag="vtsb", name="VT_sb")
                st8["qk_h0"] = ps_proj.tile([128, 512], f32, tag="ps_proj", name="qk_h0")
                st8["vq_h0"] = ps_proj.tile([128, 512], f32, tag="ps_proj", name="vq_h0")
                st8["qk_h1"] = ps_st.tile([128, 512], f32, tag="ps_st", name="qk_h1")
                st8["vq_h1"] = ps_st.tile([128, 512], f32, tag="ps_st", name="vq_h1")
            for W, ps, h in (
                (WQK, st8["qk_h0"], 0),
                (WVQ, st8["vq_h0"], 0),
                (WQK, st8["qk_h1"], 1),
                (WVQ, st8["vq_h1"], 1),
            ):
                nc.tensor.matmul(
                    ps,
                    W[:, j, :],
                    st8["xT"][:, j, ts(h, 512)],
                    start=(j == 0),
                    stop=(j == CC - 1),
                )

        def op_proj0_stage():
            st8 = states[0]
            _stage_half(st8, 0, st8.pop("qk_h0"), st8.pop("vq_h0"))
            _stage_half(st8, 1, st8.pop("qk_h1"), st8.pop("vq_h1"))
            st8.pop("xT")
            nc.gpsimd.memset(st8["VT_sb"][64:65, :], 1.0)

        def op_vp(b):
            """[V | ones] k-major blocks: Vp[p, kt, :] = [V[kt*128+p, :] 1]."""
            st8 = states[b]
            VT_sb = st8["VT_sb"]
            # D+2 stride keeps per-tile byte offsets 4-aligned (PSUM req)
            Vp = sb.tile([128, TT, D + 2], bf, tag="vp", name=f"Vp{b}")
            for g in range(2):
                # full-bank tile (2048B): sub-bank PSUM tiles can share a
                # bank with the AV accumulators, and a start=True matmul
                # clears its whole bank on HW
                pv = ps_vp.tile([128, 4, 256], bf, tag="ps_vp", name="pv")
                for u in range(4):
                    k_i = g * 4 + u
                    nc.tensor.transpose(
                        pv[:, u, 0 : D + 1],
                        VT_sb[0:65, ts(k_i, 128)],
                        ident_b[0:65, 0:65],
                    )
                nc.vector.tensor_copy(
                    Vp[:, g * 4 : (g + 1) * 4, 0 : D + 1], pv[:, :, 0 : D + 1]
                )
            st8["Vp"] = Vp

        def op_oalloc(b):
            # [128, 4, 128] f32 = exactly one 2KB bank per tile
            oA = ps_out.tile([128, 4, 128], f32, tag="ps_out", name="oA")
            oB = ps_out.tile([128, 4, 128], f32, tag="ps_out", name="oB")
            states[b]["o_AB"] = (oA, oB)

        def op_sT(b, kt):
            """S^T 512-col chunks for k-block kt + diagonal mask + exp."""
            st8 = states[b]
            QK_sb, Qd_sb = st8["QK_sb"], st8["Qd_sb"]
            E = epool.tile([128, T], bf, tag="e", name=f"E_{b}_{kt}")
            st8["E"][kt] = E
            for (q0, q1) in _seg512(kt * 128, T):
                st_c = ps_st.tile([128, 512], f32, tag="ps_st", name="st_c")
                n = q1 - q0
                nc.tensor.matmul(
                    st_c[:, 0:n],
                    QK_sb[64:128, ts(kt, 128)],
                    Qd_sb[64:128, q0:q1],
                    start=True,
                    stop=True,
                )
                if q0 == kt * 128:
                    nc.vector.tensor_add(st_c[:, 0:128], st_c[:, 0:128], dmask)
                nc.scalar.activation(
                    E[:, q0:q1],
                    st_c[:, 0:n],
                    mybir.ActivationFunctionType.Exp,
                    scale=float(SCALE),
                )

        def op_av(b, kt):
            """q-major AV: stationary = E chunk, accumulate out[q-tile, 65]."""
            st8 = states[b]
            (oA, oB), Vp = st8["o_AB"], st8["Vp"]
            E = st8["E"].pop(kt)
            for qt in range(kt, TT):
                o = oA if qt < 4 else oB
                nc.tensor.matmul(
                    o[:, qt % 4, 0 : D + 1],
                    E[:, ts(qt, 128)],
                    Vp[:, kt, 0 : D + 1],
                    start=(kt == 0 and qt in (0, 4)),
                    stop=(kt == 3 and qt == 3) or (kt == 7 and qt == 7),
                )

        def op_out_q(b, qt):
            """Normalize one q-tile straight out of PSUM."""
            st8 = states[b]
            if qt == 0:
                st8["y_sb"] = ypool.tile([128, TT, D], f32, tag="y", name="y_sb")
            oA, oB = st8["o_AB"]
            o = oA if qt < 4 else oB
            rec = sb.tile([128, 1], f32, tag="rec")
            nc.vector.reciprocal(rec, o[:, qt % 4, D : D + 1])
            nc.vector.tensor_scalar_mul(st8["y_sb"][:, qt, :], o[:, qt % 4, 0:D], rec)

        def op_out_dma(b, g):
            st8 = states[b]
            nc.gpsimd.dma_start(
                y[b].rearrange("(t p) d -> p t d", p=128)[:, 4 * g : 4 * (g + 1), :],
                st8["y_sb"][:, 4 * g : 4 * (g + 1), :],
            )
            if g == 1:
                st8.pop("o_AB")
                st8.pop("y_sb")

        # ---- pipeline schedule ----
        # steady j-slot mapping: S^T(kt) in slot JS, its AV ~2 slots later
        # (projection chunks hide the exp latency)
        JS = {0: [0], 1: [1], 2: [2, 3], 3: [4], 4: [5], 5: [6, 7]}
        JA = {2: [0], 3: [1], 4: [2, 3], 5: [4]}

        def steady_step(front_b, att_b):
            for j in range(CC):
                if att_b is not None:
                    for kt in JS.get(j, []):
                        op_sT(att_b, kt)
                if front_b is not None:
                    op_proj_h0_j(front_b, j)
                if att_b is not None:
                    for kt in JA.get(j, []):
                        op_av(att_b, kt)
            if front_b is not None:
                op_proj_h0_stage(front_b)
                op_proj_h1(front_b)
            if att_b is not None:
                op_av(att_b, 5)
                for qt in range(4):
                    op_out_q(att_b, qt)
            if front_b is not None:
                op_vp(front_b)
                op_oalloc(front_b)
            if att_b is not None:
                op_av(att_b, 6)
                op_av(att_b, 7)
                for qt in range(4, TT):
                    op_out_q(att_b, qt)

        # xT transpose queue plan (issue-rate-limited, ~1.8us each):
        #   sync:   b0 j024 | b1 j024 | b2 all | b3 j012   (15)
        #   scalar: b0 j135 | b1 j135 | [exp b0] b3 j345 | [exp b1..b3]  (9)
        for b in (0, 1):
            for j in range(CC):
                op_xt(b, j, nc.sync)
        for j in range(CC):
            op_xt(2, j, nc.sync)

        # warm-keeper junk matmuls: PE is idle until xT(0) lands; f32
        # 512-col matmuls run 4 cyc/row (~1.7us each cold) and depend only
        # on an early gpsimd memset, so they feed the HAM activity monitor
        # from ~7.5us and the first projections run at 2.4GHz.
        junk_src = const.tile([128, 512], f32, tag="junk_src")
        nc.gpsimd.memset(junk_src, 0.5)
        junk = ps_st.tile([128, 512], f32, tag="ps_st", name="junk")
        for _ in range(3):
            nc.tensor.matmul(junk, junk_src[:, 0:128], junk_src, start=True, stop=True)

        # step 0: front(0) only, both halves per chunk
        for j in range(CC):
            op_proj0_j(j)
        op_proj0_stage()
        op_vp(0)
        op_oalloc(0)
        # steps 1..3: front(s) + att(s-1); b3's transposes are emitted at
        # the head of step 2; y(b) DMA emissions are spread after the LAST
        # transpose emission so they drain during compute instead of
        # serializing on the gpsimd ring at the end
        for s in (1, 2, 3):
            if s == 2:
                for j in range(CC):
                    op_xt(3, j, nc.sync)
            if s == 3:
                op_out_dma(0, 0)
                op_out_dma(0, 1)
            steady_step(s, s - 1)
        # step 4: att(3) alone, lag-2 pipelined
        for kt in range(TT):
            op_sT(3, kt)
            if kt >= 2:
                op_av(3, kt - 2)
            if kt == 5:
                for qt in range(2):
                    op_out_q(3, qt)
        op_av(3, 6)
        for qt in range(2, 4):
            op_out_q(3, qt)
        op_out_dma(1, 0)
        op_out_dma(1, 1)
        op_out_dma(2, 0)
        op_out_dma(2, 1)
        op_av(3, 7)
        for qt in range(4, TT):
            op_out_q(3, qt)
        op_out_dma(3, 0)
        op_out_dma(3, 1)

    nc.compile()
    return nc


def _get_nc():
    if "nc" not in _cache:
        _cache["nc"] = _build()
    return _cache["nc"]


def prep_inputs(inputs):
    """Cast x to bf16 and pre-pack the projection stationaries.

    WQK[p, j, 0:64] = Wq[:, j*128+p].T, [64:128] = Wk chunk.T  (bf16)
    WVQ[p, j, 0:64] = Wv chunk.T,       [64:128] = Wq chunk.T
    """
    import ml_dtypes

    bf16 = ml_dtypes.bfloat16
    x = np.asarray(inputs["x"]).astype(bf16)
    wq = np.asarray(inputs["Wq"], dtype=np.float32)
    wk = np.asarray(inputs["Wk"], dtype=np.float32)
    wv = np.asarray(inputs["Wv"], dtype=np.float32)
    wqk = np.empty((128, CC, 128), dtype=np.float32)
    wvq = np.empty((128, CC, 128), dtype=np.float32)
    for j in range(CC):
        cs = slice(j * 128, (j + 1) * 128)
        wqk[:, j, 0:D] = wq[:, cs].T
        wqk[:, j, D:128] = wk[:, cs].T
        wvq[:, j, 0:D] = wv[:, cs].T
        wvq[:, j, D:128] = wq[:, cs].T
    return x, wqk.astype(bf16), wvq.astype(bf16)


def run(inputs, trace=False, tmpdir=None):
    """Shard, run on 8 cores, gather. Returns (y_full, BassKernelResults)."""
    from concourse.bass_utils import run_bass_kernel_spmd

    x, wqk, wvq = prep_inputs(inputs)
    assert x.shape == (B_FULL, T, C)

    nc = _get_nc()
    in_maps = [
        {
            "x": np.ascontiguousarray(x[i * B_CORE : (i + 1) * B_CORE]),
            "wqk": wqk,
            "wvq": wvq,
        }
        for i in range(N_CORES)
    ]
    kwargs = {}
    if trace:
        _install_trace_shim()
        kwargs = {"trace": True, "tmpdir": tmpdir}
    res = run_bass_kernel_spmd(nc, in_maps, list(range(N_CORES)), **kwargs)
    out = np.concatenate([res.results[i]["y"] for i in range(N_CORES)], axis=0)
    return out, res


def kernel(**inputs) -> np.ndarray:
    out, _ = run(inputs, trace=False)
    return out


def _install_trace_shim():
    """The image's antenv lacks axon_hooks; register the NTFF profile hook
    ourselves so run_bass_kernel_spmd(trace=True) works. Test-only path."""
    import types

    try:
        from antenv.axon_hooks import get_axon_ntff_profile_hook  # noqa: F401

        return
    except ImportError:
        pass
    import antenv
    from trn_agent_boot.trn_boot import _ntff_profile_via_ctypes

    mod = types.ModuleType("antenv.axon_hooks")
    mod._hook = _ntff_profile_via_ctypes("/opt/axon/libaxon_pjrt.so")
    mod.set_axon_ntff_profile_hook = lambda h: setattr(mod, "_hook", h)
    mod.get_axon_ntff_profile_hook = lambda: mod._hook
    sys.modules["antenv.axon_hooks"] = mod
    antenv.axon_hooks = mod

    import concourse.bass_utils as bu

    bu.upload_artifacts = lambda tmpdir: tmpdir
